# revision 4
# baseline (speedup 1.0000x reference)
"""Trainium2 Bass kernel for GNN mean aggregation (nn_AggrGSMean).

Computes, for t in {0,1}:
    out_t[b, v, :] = segment_sum(features_t over edges with dest v) / degree[b, v, t]
where degree[b, v, t] = max(count(adjacency[b, v, t, :] >= 0), 1).

Strategy (graph-partition sharding, per the problem's sharding hint):
- Host: partition edges by destination-vertex range across 8 cores, sort each
  core's edges by destination, group into 128-vertex blocks, pad each block's
  edge list to a fixed number of 128-edge tiles (T_BLK, global max so the SPMD
  program is uniform).  Each edge row carries 65 floats: 64 features + its
  vertex-slot-in-block (or -1 for padding).
- Device (per core): for each vertex block, stream edge tiles; build a one-hot
  [128 edges x 128 vslots] matrix on DVE/GPSIMD (iota == vslot), and
  matmul-accumulate onehot.T @ features into PSUM -> the per-block segment sum.
  Degree is computed from the adjacency slice on-chip (compare/reduce), and the
  mean division is fused into the PSUM->SBUF evacuation on the Scalar engine
  (activation Copy with per-partition scale = 1/degree).
"""

import sys

if "/opt/trn_rl_repo" not in sys.path:
    sys.path.insert(0, "/opt/trn_rl_repo")

import numpy as np

# Problem constants (hardcoded per contract)
B, V, T, N, F, M = 1, 100000, 2, 32, 64, 1600000
NCORES = 8
BLK = 128           # vertices per block == matmul one-hot width
FW = F + 1          # 64 features + 1 vloc slot per edge row


class Cfg:
    def __init__(self, v=V, ncores=NCORES):
        self.V = v
        self.NCORES = ncores
        self.VLOC = v // ncores
        self.NBLK = (self.VLOC + BLK - 1) // BLK
        self.VPAD = self.NBLK * BLK


_DEFAULT_CFG = Cfg()
_NC_CACHE = {}


def build_device_program(t_blk, cfg=_DEFAULT_CFG, gpsimd_frac=0.5):
    """Build + compile the per-core Bass program (same program on all cores)."""
    from contextlib import ExitStack

    import concourse.tile as tile
    from concourse import bacc, mybir

    f32 = mybir.dt.float32
    i32 = mybir.dt.int32
    NBLK = cfg.NBLK

    nc = bacc.Bacc("TRN2", target_bir_lowering=False, debug=False)
    feat_d = [
        nc.dram_tensor(f"feat{t}", [NBLK, BLK, t_blk * FW], f32, kind="ExternalInput").ap()
        for t in range(T)
    ]
    # adjacency grouped so each partition reads ADJ_G*256B contiguously
    ADJ_G = 7  # 98 = 14 * 7 block groups
    assert NBLK % ADJ_G == 0
    adj_d = nc.dram_tensor("adj", [NBLK // ADJ_G, BLK, ADJ_G * T * N], i32, kind="ExternalInput").ap()
    iota_d = nc.dram_tensor("iota", [BLK, BLK], f32, kind="ExternalInput").ap()
    out_d = nc.dram_tensor("out", [NBLK, BLK, T * F], f32, kind="ExternalOutput").ap()

    with tile.TileContext(nc) as tc, ExitStack() as ctx:
        const = ctx.enter_context(tc.tile_pool(name="const", bufs=1))
        featp = ctx.enter_context(tc.tile_pool(name="featp", bufs=4))
        adjp = ctx.enter_context(tc.tile_pool(name="adjp", bufs=2))
        degp = ctx.enter_context(tc.tile_pool(name="degp", bufs=3))
        ohp = ctx.enter_context(tc.tile_pool(name="ohp", bufs=6))
        outp = ctx.enter_context(tc.tile_pool(name="outp", bufs=3))
        psump = ctx.enter_context(tc.tile_pool(name="psum", bufs=4, space="PSUM"))

        iota_t = const.tile([BLK, BLK], f32)
        nc.sync.dma_start(out=iota_t[:], in_=iota_d[:])

        # one-hot engine schedule: route gpsimd_frac of one-hot builds to GPSIMD
        def oh_engine(seq):
            take_gp = int((seq + 1) * gpsimd_frac) > int(seq * gpsimd_frac)
            return nc.gpsimd if take_gp else nc.vector

        oh_seq = 0
        for bg in range(NBLK // ADJ_G):
            adj_t = adjp.tile([BLK, ADJ_G * T * N], i32)
            nc.sync.dma_start(out=adj_t[:], in_=adj_d[bg])
            val = degp.tile([BLK, ADJ_G * T * N], f32, tag="val")
            nc.vector.tensor_scalar(
                val[:], adj_t[:], 0, None, op0=mybir.AluOpType.is_ge
            )
            deg = degp.tile([BLK, ADJ_G * T], f32, tag="deg")
            nc.vector.tensor_reduce(
                deg[:],
                val[:].rearrange("p (g n) -> p g n", n=N),
                axis=mybir.AxisListType.X,
                op=mybir.AluOpType.add,
            )
            rec = degp.tile([BLK, ADJ_G * T], f32, tag="rec")
            nc.vector.tensor_scalar(
                deg[:], deg[:], 1.0, None, op0=mybir.AluOpType.max
            )
            nc.vector.reciprocal(rec[:], deg[:])

            for bo in range(ADJ_G):
                b = bg * ADJ_G + bo
                out_t = outp.tile([BLK, T * F], f32)
                for t in range(T):
                    feat_t = featp.tile([BLK, t_blk * FW], f32)
                    nc.sync.dma_start(out=feat_t[:], in_=feat_d[t][b])
                    ps = psump.tile([BLK, F], f32)
                    for i in range(t_blk):
                        oh = ohp.tile([BLK, BLK], f32)
                        eng = oh_engine(oh_seq)
                        oh_seq += 1
                        eng.tensor_scalar(
                            oh[:],
                            iota_t[:],
                            feat_t[:, i * FW + F : i * FW + F + 1],
                            None,
                            op0=mybir.AluOpType.is_equal,
                        )
                        nc.tensor.matmul(
                            ps[:],
                            lhsT=oh[:],
                            rhs=feat_t[:, i * FW : i * FW + F],
                            start=(i == 0),
                            stop=(i == t_blk - 1),
                        )
                    # mean = psum * (1/deg); fused into PSUM evacuation on ScalarE
                    nc.scalar.mul(
                        out_t[:, t * F : (t + 1) * F],
                        ps[:],
                        rec[:, bo * T + t : bo * T + t + 1],
                    )
                nc.sync.dma_start(out=out_d[b], in_=out_t[:])

    nc.compile()
    return nc


def shard_table(indices, cfg=_DEFAULT_CFG):
    """Sort edges by destination, partition by core, group into blocks.

    Returns (per_core list of (orig_edge_idx, block, rank_in_block, counts),
             max tiles-per-block over this table).
    """
    v = np.ascontiguousarray(indices[:, 1])
    order = np.argsort(v, kind="stable")
    vs = v[order]
    bounds = np.searchsorted(vs, np.arange(cfg.NCORES + 1) * cfg.VLOC)
    per_core = []
    t_blk_max = 1
    for c in range(cfg.NCORES):
        lo, hi = bounds[c], bounds[c + 1]
        idx = order[lo:hi]
        vloc = vs[lo:hi].astype(np.int64) - c * cfg.VLOC
        blk = vloc >> 7  # // 128
        vin = vloc & 127
        cnt = np.bincount(blk, minlength=cfg.NBLK).astype(np.int64)
        starts = np.zeros(cfg.NBLK, dtype=np.int64)
        np.cumsum(cnt[:-1], out=starts[1:])
        rank = np.arange(len(idx), dtype=np.int64) - starts[blk]
        per_core.append((idx, blk, vin, rank))
        if len(cnt):
            t_blk_max = max(t_blk_max, int((cnt.max() + BLK - 1) // BLK))
    return per_core, t_blk_max


def fill_feature_stream(per_core, features, t_blk, cfg=_DEFAULT_CFG):
    """Build per-core padded edge stream [NBLK, BLK(edge-slot), T_BLK*FW]."""
    out = np.zeros((cfg.NCORES, cfg.NBLK * BLK * t_blk, FW), dtype=np.float32)
    out[:, :, F] = -1.0
    for c in range(cfg.NCORES):
        idx, blk, vin, rank = per_core[c]
        # slot layout: [block, edge_slot(=rank%128), tile(=rank//128), FW]
        slot = blk * (BLK * t_blk) + (rank & 127) * t_blk + (rank >> 7)
        out[c, slot, :F] = features[idx]
        out[c, slot, F] = vin
    return out.reshape(cfg.NCORES, cfg.NBLK, BLK, t_blk * FW)


def prep_adjacency(adjacency, adj_g=7, cfg=_DEFAULT_CFG):
    adj = np.ascontiguousarray(adjacency.reshape(cfg.V, T * N))
    adj_pad = np.full((cfg.NCORES, cfg.VPAD, T * N), -1, dtype=np.int32)
    adj_pad[:, : cfg.VLOC] = adj.reshape(cfg.NCORES, cfg.VLOC, T * N)
    # regroup: [NBLK/G, BLK, G*T*N] with per-partition contiguous G*T*N chunk
    a = adj_pad.reshape(cfg.NCORES, cfg.NBLK // adj_g, adj_g, BLK, T * N)
    a = np.ascontiguousarray(a.transpose(0, 1, 3, 2, 4))
    return a.reshape(cfg.NCORES, cfg.NBLK // adj_g, BLK, adj_g * T * N)


def prepare_inputs(adjacency, indices0, features0, indices1, features1, cfg=_DEFAULT_CFG):
    adjacency = np.asarray(adjacency)
    per_core0, tb0 = shard_table(np.asarray(indices0), cfg)
    per_core1, tb1 = shard_table(np.asarray(indices1), cfg)
    t_blk = max(tb0, tb1)

    f0 = fill_feature_stream(per_core0, np.asarray(features0, dtype=np.float32), t_blk, cfg)
    f1 = fill_feature_stream(per_core1, np.asarray(features1, dtype=np.float32), t_blk, cfg)
    adj = prep_adjacency(adjacency, 7, cfg)
    iota = np.broadcast_to(np.arange(BLK, dtype=np.float32), (BLK, BLK)).copy()

    in_maps = [
        {"feat0": f0[c], "feat1": f1[c], "adj": adj[c], "iota": iota}
        for c in range(cfg.NCORES)
    ]
    return in_maps, t_blk


def assemble_output(core_outs, cfg=_DEFAULT_CFG):
    outs = []
    for t in range(T):
        parts = [
            core_outs[c].reshape(cfg.VPAD, T, F)[: cfg.VLOC, t, :]
            for c in range(cfg.NCORES)
        ]
        outs.append(np.concatenate(parts, axis=0).reshape(B, cfg.V, F))
    return (outs[0], outs[1])


def kernel(adjacency, indices0, features0, indices1, features1):
    from concourse.bass_utils import run_bass_kernel_spmd

    cfg = _DEFAULT_CFG
    in_maps, t_blk = prepare_inputs(
        adjacency, indices0, features0, indices1, features1, cfg
    )

    if t_blk not in _NC_CACHE:
        _NC_CACHE[t_blk] = build_device_program(t_blk, cfg)
    nc = _NC_CACHE[t_blk]

    res = run_bass_kernel_spmd(nc, in_maps, list(range(cfg.NCORES)))
    return assemble_output(
        [res.results[c]["out"] for c in range(cfg.NCORES)], cfg
    )


# revision 6
# speedup vs baseline: 5.8696x; 5.8696x over previous
"""Trainium2 Bass kernel for GNN mean aggregation (nn_AggrGSMean).

Computes, for t in {0,1}:
    out_t[b, v, :] = segment_sum(features_t over edges with dest v) / degree[b, v, t]
where degree[b, v, t] = max(count(adjacency[b, v, t, :] >= 0), 1).

Strategy (graph-partition sharding per the problem's sharding hint):
- Host: partition edges by destination-vertex range across 8 cores, sort each
  core's edges by destination, group into 128-vertex blocks.  Each block's edge
  list is padded to a whole number of 128-edge tiles.  Blocks are assigned to
  "slots" in decreasing-tile-count order so one static per-slot tile profile
  (max over cores/tables at each rank) serves all cores with ~8% less padding
  than a uniform max.  Features ship as bf16 hi+lo halves (their sum is the
  fp32 value to ~1e-5) plus the destination slot-in-block encoded as a float.
- Device (per core): for each slot, stream 128-edge tiles [hi64|lo64|negv]
  bf16; build a one-hot [128 edges x 128 vslots] in bf16 (iota == vslot) on
  DVE (a fraction on ScalarE via relu(1-(iota-v)^2)); one matmul per tile
  accumulates onehot.T @ [hi|lo] into PSUM [128, 128].  Degree comes from the
  adjacency slice on-chip; the hi/lo halves are summed by a strided
  tensor_reduce and the mean division rides the ScalarE copy (per-partition
  scale = 1/degree).
"""

import sys

if "/opt/trn_rl_repo" not in sys.path:
    sys.path.insert(0, "/opt/trn_rl_repo")

import ml_dtypes
import numpy as np

# Problem constants (hardcoded per contract)
B, V, T, N, F, M = 1, 100000, 2, 32, 64, 1600000
NCORES = 8
BLK = 128
EW = 2 * F + 2  # bf16 words per edge row: 64 hi + 64 lo + 1 f32 (=2 bf16) negv
ADJ_G = 7

ONE_F32_U16 = np.array([0x0000, 0x3F80], dtype=np.uint16)  # f32 1.0 as 2 LE u16


class Cfg:
    def __init__(self, v=V, ncores=NCORES):
        self.V = v
        self.NCORES = ncores
        self.VLOC = v // ncores
        self.NBLK = (self.VLOC + BLK - 1) // BLK
        self.VPAD = self.NBLK * BLK


_DEFAULT_CFG = Cfg()
_NC_CACHE = {}


def build_device_program(profile, cfg=_DEFAULT_CFG, act_frac=0.14):
    """Build + compile the per-core Bass program.

    profile: per-slot tile counts (len NBLK); same static schedule on all cores.
    act_frac: fraction of one-hot builds routed to ScalarE (2-op trick) to
    offload the Vector engine.
    """
    from contextlib import ExitStack

    import concourse.tile as tile
    from concourse import bacc, mybir

    f32 = mybir.dt.float32
    bf16 = mybir.dt.bfloat16
    i32 = mybir.dt.int32
    NBLK = cfg.NBLK
    assert len(profile) == NBLK and NBLK % ADJ_G == 0
    t_max = max(profile)
    slot_elems = [BLK * ts * EW for ts in profile]
    slot_base = np.concatenate([[0], np.cumsum(slot_elems)]).astype(np.int64)
    total_elems = int(slot_base[-1])

    nc = bacc.Bacc("TRN2", target_bir_lowering=False, debug=False)
    feat_d = [
        nc.dram_tensor(f"feat{t}", [total_elems], bf16, kind="ExternalInput").ap()
        for t in range(T)
    ]
    adj_d = nc.dram_tensor(
        "adj", [NBLK // ADJ_G, BLK, ADJ_G * T * N], i32, kind="ExternalInput"
    ).ap()
    # iota_neg[e, j] = -j (f32) for DVE is_equal against negv;
    # iota_pos[e, j] = +j (bf16) for the ScalarE (j + negv)^2 path
    iota_n_d = nc.dram_tensor("iota_neg", [BLK, BLK], f32, kind="ExternalInput").ap()
    iota_p_d = nc.dram_tensor("iota_pos", [BLK, BLK], bf16, kind="ExternalInput").ap()
    out_d = nc.dram_tensor("out", [NBLK, BLK, T * F], f32, kind="ExternalOutput").ap()

    with tile.TileContext(nc) as tc, ExitStack() as ctx:
        const = ctx.enter_context(tc.tile_pool(name="const", bufs=1))
        featp = ctx.enter_context(tc.tile_pool(name="featp", bufs=4))
        adjp = ctx.enter_context(tc.tile_pool(name="adjp", bufs=2))
        degp = ctx.enter_context(tc.tile_pool(name="degp", bufs=3))
        ohp = ctx.enter_context(tc.tile_pool(name="ohp", bufs=8))
        redp = ctx.enter_context(tc.tile_pool(name="redp", bufs=3))
        outp = ctx.enter_context(tc.tile_pool(name="outp", bufs=3))
        psump = ctx.enter_context(tc.tile_pool(name="psum", bufs=4, space="PSUM"))

        iota_n = const.tile([BLK, BLK], f32)
        nc.sync.dma_start(out=iota_n[:], in_=iota_n_d[:])
        iota_p = const.tile([BLK, BLK], bf16)
        nc.sync.dma_start(out=iota_p[:], in_=iota_p_d[:])

        oh_seq = 0

        def build_onehot(oh, negv_ap):
            nonlocal oh_seq
            use_act = int((oh_seq + 1) * act_frac) > int(oh_seq * act_frac)
            oh_seq += 1
            if use_act:
                y = ohp.tile([BLK, BLK], bf16, tag="y")
                nc.scalar.activation(
                    y[:], iota_p[:], mybir.ActivationFunctionType.Square,
                    bias=negv_ap, scale=1.0,
                )
                nc.scalar.activation(
                    oh[:], y[:], mybir.ActivationFunctionType.Relu,
                    bias=1.0, scale=-1.0,
                )
            else:
                nc.vector.tensor_scalar(
                    oh[:], iota_n[:], negv_ap, None, op0=mybir.AluOpType.is_equal
                )

        for bg in range(NBLK // ADJ_G):
            adj_t = adjp.tile([BLK, ADJ_G * T * N], i32)
            nc.sync.dma_start(out=adj_t[:], in_=adj_d[bg])
            val = degp.tile([BLK, ADJ_G * T * N], f32, tag="val")
            nc.vector.tensor_scalar(
                val[:], adj_t[:], 0, None, op0=mybir.AluOpType.is_ge
            )
            deg = degp.tile([BLK, ADJ_G * T], f32, tag="deg")
            nc.vector.tensor_reduce(
                deg[:],
                val[:].rearrange("p (g n) -> p g n", n=N),
                axis=mybir.AxisListType.X,
                op=mybir.AluOpType.add,
            )
            rec = degp.tile([BLK, ADJ_G * T], f32, tag="rec")
            nc.vector.tensor_scalar(
                deg[:], deg[:], 1.0, None, op0=mybir.AluOpType.max
            )
            nc.vector.reciprocal(rec[:], deg[:])

            for bo in range(ADJ_G):
                s = bg * ADJ_G + bo
                t_s = profile[s]
                out_t = outp.tile([BLK, T * F], f32)
                for t in range(T):
                    feat_t = featp.tile([BLK, t_max * EW], bf16, tag="feat")
                    src = feat_d[t][
                        int(slot_base[s]) : int(slot_base[s + 1])
                    ].rearrange("(e w) -> e w", w=t_s * EW)
                    nc.sync.dma_start(out=feat_t[:, : t_s * EW], in_=src)
                    ps = psump.tile([BLK, 2 * F], f32)
                    for i in range(t_s):
                        oh = ohp.tile([BLK, BLK], bf16, tag="oh")
                        negv = feat_t[:, i * EW + 2 * F : i * EW + 2 * F + 2].bitcast(f32)
                        build_onehot(oh, negv)
                        nc.tensor.matmul(
                            ps[:],
                            lhsT=oh[:],
                            rhs=feat_t[:, i * EW : i * EW + 2 * F],
                            start=(i == 0),
                            stop=(i == t_s - 1),
                        )
                    # sum hi+lo halves: [128, (2,64)] -> [128, 64]
                    red = redp.tile([BLK, F], f32)
                    nc.vector.tensor_reduce(
                        red[:],
                        ps[:].rearrange("p (h f) -> p f h", h=2),
                        axis=mybir.AxisListType.X,
                        op=mybir.AluOpType.add,
                    )
                    # mean = sum * (1/deg) on ScalarE
                    nc.scalar.mul(
                        out_t[:, t * F : (t + 1) * F],
                        red[:],
                        rec[:, bo * T + t : bo * T + t + 1],
                    )
                nc.sync.dma_start(out=out_d[s], in_=out_t[:])

    nc.compile()
    return nc


def shard_table(indices, cfg=_DEFAULT_CFG):
    """Sort edges by destination and partition by core.

    Returns per-core list of (orig_edge_idx sorted by dest, block, rank_in_block,
    tiles_per_block)."""
    v = np.ascontiguousarray(indices[:, 1])
    order = np.argsort(v, kind="stable")
    vs = v[order]
    bounds = np.searchsorted(vs, np.arange(cfg.NCORES + 1) * cfg.VLOC)
    per_core = []
    for c in range(cfg.NCORES):
        lo, hi = bounds[c], bounds[c + 1]
        idx = order[lo:hi]
        vloc = vs[lo:hi].astype(np.int64) - c * cfg.VLOC
        blk = vloc >> 7
        vin = vloc & 127
        cnt = np.bincount(blk, minlength=cfg.NBLK).astype(np.int64)
        starts = np.zeros(cfg.NBLK, dtype=np.int64)
        np.cumsum(cnt[:-1], out=starts[1:])
        rank = np.arange(len(idx), dtype=np.int64) - starts[blk]
        tiles = (cnt + BLK - 1) // BLK
        per_core.append((idx, blk, vin, rank, tiles))
    return per_core


def make_profile(per_core_tables, cfg=_DEFAULT_CFG):
    """Slot tile profile + per (core, table) block->slot permutation."""
    perms = []  # perms[t][c] = array: slot -> block
    sorted_tiles = []
    for per_core in per_core_tables:
        perms_t = []
        for c in range(cfg.NCORES):
            tiles = per_core[c][4]
            order = np.argsort(-tiles, kind="stable")
            perms_t.append(order)
            sorted_tiles.append(tiles[order])
        perms.append(perms_t)
    profile = np.max(np.stack(sorted_tiles), axis=0)
    profile = np.maximum(profile, 1)
    return [int(x) for x in profile], perms


def fill_feature_stream(per_core, features, profile, perm_t, cfg=_DEFAULT_CFG):
    """Per-core bf16 edge stream, slot-major, edge-slot-major within a slot.

    Row layout (130 bf16 words): [hi(64) | lo(64) | negv as f32 (2 words)].
    Padding rows have negv = +1.0 (never matches iota_neg <= 0)."""
    prof = np.asarray(profile, dtype=np.int64)
    row_base = np.concatenate([[0], np.cumsum(prof * BLK)]).astype(np.int64)
    total_rows = int(row_base[-1])

    hi = features.astype(ml_dtypes.bfloat16)
    lo = (features - hi.astype(np.float32)).astype(ml_dtypes.bfloat16)
    hi_u = hi.view(np.uint16)
    lo_u = lo.view(np.uint16)

    out = np.zeros((cfg.NCORES, total_rows, EW), dtype=np.uint16)
    out[:, :, 2 * F :] = ONE_F32_U16  # negv = +1.0 for padding rows
    for c in range(cfg.NCORES):
        idx, blk, vin, rank, _tiles = per_core[c]
        inv = np.empty(cfg.NBLK, dtype=np.int64)
        inv[perm_t[c]] = np.arange(cfg.NBLK)
        s = inv[blk]
        rows = row_base[s] + (rank & 127) * prof[s] + (rank >> 7)
        out[c, rows, 0:F] = hi_u[idx]
        out[c, rows, F : 2 * F] = lo_u[idx]
        out[c, rows, 2 * F :] = (
            (-vin.astype(np.float32)).view(np.uint32).view(np.uint16).reshape(-1, 2)
        )
    return out.reshape(cfg.NCORES, total_rows * EW).view(ml_dtypes.bfloat16)


def prep_adjacency(adjacency, perms, cfg=_DEFAULT_CFG):
    """adj_dev[c, g, vin, j*64 + t*32 + n] = adjacency[0, block_{t}(c, 7g+j), vin, t, n]
    padded with -1 beyond VLOC."""
    adj = np.ascontiguousarray(adjacency.reshape(cfg.V, T, N))
    adj_pad = np.full((cfg.NCORES, cfg.VPAD, T, N), -1, dtype=np.int32)
    adj_pad[:, : cfg.VLOC] = adj.reshape(cfg.NCORES, cfg.VLOC, T, N)
    adj_pad = adj_pad.reshape(cfg.NCORES, cfg.NBLK, BLK, T, N)
    out = np.empty((cfg.NCORES, cfg.NBLK, BLK, T, N), dtype=np.int32)
    for c in range(cfg.NCORES):
        for t in range(T):
            out[c, :, :, t, :] = adj_pad[c, perms[t][c], :, t, :]
    # [c, g, j, vin, t, n] -> [c, g, vin, j, t, n]
    out = out.reshape(cfg.NCORES, cfg.NBLK // ADJ_G, ADJ_G, BLK, T * N)
    out = np.ascontiguousarray(out.transpose(0, 1, 3, 2, 4))
    return out.reshape(cfg.NCORES, cfg.NBLK // ADJ_G, BLK, ADJ_G * T * N)


def prepare_inputs(adjacency, indices0, features0, indices1, features1, cfg=_DEFAULT_CFG):
    adjacency = np.asarray(adjacency)
    pc0 = shard_table(np.asarray(indices0), cfg)
    pc1 = shard_table(np.asarray(indices1), cfg)
    profile, perms = make_profile([pc0, pc1], cfg)

    f0 = fill_feature_stream(
        pc0, np.asarray(features0, dtype=np.float32), profile, perms[0], cfg
    )
    f1 = fill_feature_stream(
        pc1, np.asarray(features1, dtype=np.float32), profile, perms[1], cfg
    )
    adj = prep_adjacency(adjacency, perms, cfg)
    iota_neg = np.broadcast_to(
        -np.arange(BLK, dtype=np.float32), (BLK, BLK)
    ).copy()
    iota_pos = np.broadcast_to(
        np.arange(BLK).astype(ml_dtypes.bfloat16), (BLK, BLK)
    ).copy()

    in_maps = [
        {
            "feat0": f0[c],
            "feat1": f1[c],
            "adj": adj[c],
            "iota_neg": iota_neg,
            "iota_pos": iota_pos,
        }
        for c in range(cfg.NCORES)
    ]
    return in_maps, profile, perms


def assemble_output(core_outs, perms, cfg=_DEFAULT_CFG):
    outs = []
    for t in range(T):
        parts = []
        for c in range(cfg.NCORES):
            res_t = core_outs[c].reshape(cfg.NBLK, BLK, T, F)[:, :, t, :]
            tmp = np.empty((cfg.NBLK, BLK, F), dtype=res_t.dtype)
            tmp[perms[t][c]] = res_t
            parts.append(tmp.reshape(cfg.VPAD, F)[: cfg.VLOC])
        outs.append(np.concatenate(parts, axis=0).reshape(B, cfg.V, F))
    return (outs[0], outs[1])


def kernel(adjacency, indices0, features0, indices1, features1):
    from concourse.bass_utils import run_bass_kernel_spmd

    cfg = _DEFAULT_CFG
    in_maps, profile, perms = prepare_inputs(
        adjacency, indices0, features0, indices1, features1, cfg
    )

    key = tuple(profile)
    if key not in _NC_CACHE:
        _NC_CACHE[key] = build_device_program(profile, cfg)
    nc = _NC_CACHE[key]

    res = run_bass_kernel_spmd(nc, in_maps, list(range(cfg.NCORES)))
    return assemble_output(
        [res.results[c]["out"] for c in range(cfg.NCORES)], perms, cfg
    )
